# revision 3
# baseline (speedup 1.0000x reference)
"""Trainium2 Bass kernel v2 for nn_EquivariantModel (e3nn-style equivariant net).

Data-parallel over batch (8 cores x 1024 rows), feature-major activations
[feature, batch].  Per-block tensor product z[(u,v),b] = f1[u,b]*f2[v,b]
is formed with:
  - partition-broadcast of f1 rows via DMA from a DRAM staging copy
    (frees PE + scalar engine from broadcast work),
  - grouped DVE f16 multiplies (2 fused multi-dim-AP tensor_tensor ops
    per kt, 2x perf mode; GPSIMD deliberately unused - it shares an SBUF
    port with DVE and slows the 2-port DVE muls down),
  - PSUM-accumulated matmuls over k-tiles with redundant Ldweights
    elided (ins.ldweights=False) within same-weight runs.
Out-linears are folded into the next block's l1/l2 linears host-side.
"""

import sys
import numpy as np

if '/opt/trn_rl_repo' not in sys.path:
    sys.path.insert(0, '/opt/trn_rl_repo')

B, M_IN, M_HID = 8192, 64, 128
N_CORES = 8
BC = B // N_CORES            # batch per core
CH = 512                     # chunk of batch per matmul / mul group
NCH = BC // CH
TANH_GAIN = 1.5927116870880127
G_W = 8                      # b2 weight kts per stream DMA
LDW_SKIP = True              # skip redundant weight loads within a weight run

_CACHE = {}


def _build_program(repeat=1):
    import concourse.mybir as mybir
    import concourse.tile as tile
    from concourse import bacc
    from contextlib import ExitStack

    f32 = mybir.dt.float32
    f16 = mybir.dt.float16
    Tanh = mybir.ActivationFunctionType.Tanh

    nc = bacc.Bacc("TRN2", target_bir_lowering=False)

    # ---- DRAM I/O ----
    s0 = nc.dram_tensor("s0", [64, BC], f16, kind="ExternalInput")
    v0 = nc.dram_tensor("v0", [192, BC], f16, kind="ExternalInput")

    # TP weights, k-major concatenated: [128, KT*4*128] f16
    wtp1_d = nc.dram_tensor("wtp1", [128, 32 * 4 * 128], f16, kind="ExternalInput")
    wtp2_d = nc.dram_tensor("wtp2", [128, 128 * 4 * 128], f16, kind="ExternalInput")

    # small linear / gate weights
    lw = {}
    for nm, shp in (
        ("l1_s", (64, 64)), ("l1_v", (64, 64)), ("l2_s", (64, 64)), ("l2_v", (64, 64)),
        ("g1_ws", (128, 128)), ("g1_wg", (128, 128)), ("g1_wv", (128, 128)),
        ("f2_l1_s", (128, 128)), ("f2_l1_v", (128, 128)),
        ("f2_l2_s", (128, 128)), ("f2_l2_v", (128, 128)),
        ("g2_ws", (128, 128)), ("g2_wg", (128, 128)), ("g2_wv", (128, 128)),
        ("ff_s", (128, 64)), ("ff_v", (128, 64)),
    ):
        lw[nm] = nc.dram_tensor(nm, list(shp), f16, kind="ExternalInput")

    # DRAM staging for broadcast factors [U, 4*BC]
    f1d = {
        "b1": nc.dram_tensor("f1d_b1", [64, 4 * BC], f16, kind="Internal"),
        "b2": nc.dram_tensor("f1d_b2", [128, 4 * BC], f16, kind="Internal"),
    }

    out_d = nc.dram_tensor("out", [256, BC], f32, kind="ExternalOutput")

    with ExitStack() as ctx:
        tc = ctx.enter_context(tile.TileContext(nc))
        consts = ctx.enter_context(tc.tile_pool(name="consts", bufs=1))
        acts = ctx.enter_context(tc.tile_pool(name="acts", bufs=1))
        wstream = ctx.enter_context(tc.tile_pool(name="wstream", bufs=2))
        bc_pool = ctx.enter_context(tc.tile_pool(name="bcast", bufs=3))
        zA_pool = ctx.enter_context(tc.tile_pool(name="zA", bufs=2))
        zB_pool = ctx.enter_context(tc.tile_pool(name="zBC", bufs=2))
        # single PSUM pool: 8 banks as 8 rotating [128, 512] tags
        psum = ctx.enter_context(tc.tile_pool(name="psum", bufs=1, space="PSUM"))

        def pstile(tag):
            return psum.tile([128, CH], f32, tag=tag, name=tag)

        # ---- constants ----
        W = {}
        for nm, t in lw.items():
            w = consts.tile(list(t.shape), t.dtype, tag=nm, name=nm)
            nc.sync.dma_start(w[:], t[:])
            W[nm] = w
        wb1 = consts.tile([128, 32 * 4 * 128], f16, tag="wb1", name="wb1")
        nc.sync.dma_start(wb1[:], wtp1_d[:])

        # ---- input activations ----
        sT = acts.tile([64, BC], f16, tag="in_s", name="in_s")
        nc.sync.dma_start(sT[:], s0[:])
        vT = []
        for i in range(3):
            t = acts.tile([64, BC], f16, tag=f"in_v{i}", name=f"in_v{i}")
            nc.sync.dma_start(t[:], v0[i * 64:(i + 1) * 64, :])
            vT.append(t)

        def tp_block(blk, U, KT, f1_all, f2_all, wtile_fn):
            """TP loop (kt outer, full BC per kt): returns tp_s, tp_v f16."""
            tp_s = acts.tile([128, BC], f16, tag="tp_s", name="tp_s")
            tp_v = acts.tile([128, 3 * BC], f16, tag="tp_v", name="tp_v")
            tp_vv = tp_v[:].rearrange("p (s f) -> p s f", s=3)

            f2v = f2_all[:].rearrange("p (s f) -> p s f", s=4)  # [128,4,BC]

            # accs: [quantity][chunk] -> [128, CH] psum tile
            acc = {(q, c): pstile(f"acc{q}{c}")
                   for q in range(4) for c in range(NCH)}
            chunks = [(c, slice(c * CH, (c + 1) * CH)) for c in range(NCH)]
            for kt in range(KT):
                # broadcast tile [128, 4, BC] from DRAM staging
                bcast = bc_pool.tile([128, 4 * BC], f16, tag="bc", name="bc")
                bcv = bcast[:].rearrange("p (s f) -> p s f", s=4)
                if U == 128:
                    nc.sync.dma_start(
                        bcast[:], f1d[blk][kt:kt + 1, :].partition_broadcast(128))
                else:
                    nc.sync.dma_start(
                        bcast[0:64, :],
                        f1d[blk][2 * kt:2 * kt + 1, :].partition_broadcast(64))
                    nc.sync.dma_start(
                        bcast[64:128, :],
                        f1d[blk][2 * kt + 1:2 * kt + 2, :].partition_broadcast(64))

                # group A: {ss, sv0, sv1, sv2} = bc_s * (s2, v2_i)
                zA = zA_pool.tile([128, 4 * BC], f16, tag="zA", name="zA")
                zAv = zA[:].rearrange("p (s f) -> p s f", s=4)
                nc.vector.tensor_mul(
                    zAv, bcv[:, 0:1, :].to_broadcast((128, 4, BC)), f2v)
                # group C: {vv0..2} = bc_v_i * v2_i ; group B: {vs0..2}
                zBC = zB_pool.tile([128, 6 * BC], f16, tag="zBC", name="zBC")
                zBCv = zBC[:].rearrange("p (a s f) -> p a s f", a=2, s=3)
                nc.vector.tensor_mul(zBCv[:, 1, :, :], bcv[:, 1:4, :],
                                     f2v[:, 1:4, :])
                nc.vector.tensor_mul(zBCv[:, 0, :, :], bcv[:, 1:4, :],
                                     f2v[:, 0:1, :].to_broadcast((128, 3, BC)))

                wss, wvv, wsv, wvs = wtile_fn(kt)
                first, last = kt == 0, kt == KT - 1

                def bc_sl(a, i, c):
                    off = (a * 3 + i) * BC + c * CH
                    return zBC[:, off:off + CH]

                # weight-outer runs; skip redundant weight loads within a run
                runs = (
                    [(wss, [(acc[0, c], zAv[:, 0, sl], first, False)
                            for c, sl in chunks])] +
                    [(wsv, [(acc[1 + i, c], zAv[:, 1 + i, sl], first, False)
                            for i in range(3) for c, sl in chunks])] +
                    [(wvv, [(acc[0, c], bc_sl(1, i, c), False, last and i == 2)
                            for i in range(3) for c, sl in chunks])] +
                    [(wvs, [(acc[1 + i, c], bc_sl(0, i, c), False, last)
                            for i in range(3) for c, sl in chunks])]
                )
                for w, mms in runs:
                    for j, (dst, src, st, sp) in enumerate(mms):
                        inst = nc.tensor.matmul(dst, w, src, start=st, stop=sp)
                        if LDW_SKIP and j > 0:
                            inst.ins.ldweights = False

            for c in range(NCH):
                sl = slice(c * CH, (c + 1) * CH)
                nc.scalar.copy(tp_s[:, sl], acc[0, c][:])
                for i in range(3):
                    nc.scalar.copy(tp_vv[:, i, sl], acc[1 + i, c][:])
            return tp_s, tp_v, tp_vv

        def gate(gws, gwg, gwv, tp_s, tp_vv):
            """-> tanh_s [128,BC] f16, gated [128,3*BC] f16 (grouped)."""
            tanh_s = acts.tile([128, BC], f16, tag="tanh_s", name="tanh_s")
            tg = acts.tile([128, BC], f16, tag="tg", name="tg")
            gv = acts.tile([128, 3 * BC], f16, tag="gv", name="gv")
            gvv = gv[:].rearrange("p (s f) -> p s f", s=3)
            gated = acts.tile([128, 3 * BC], f16, tag="gated", name="gated")
            gatedv = gated[:].rearrange("p (s f) -> p s f", s=3)
            chunks = [(c, slice(c * CH, (c + 1) * CH)) for c in range(NCH)]
            for c, sl in chunks:
                p1 = pstile(f"acc0{c}")
                inst = nc.tensor.matmul(p1, gws, tp_s[:, sl],
                                        start=True, stop=True)
                if LDW_SKIP and c > 0:
                    inst.ins.ldweights = False
                nc.scalar.activation(tanh_s[:, sl], p1[:], Tanh)
            for c, sl in chunks:
                p2 = pstile(f"acc1{c}")
                inst = nc.tensor.matmul(p2, gwg, tp_s[:, sl],
                                        start=True, stop=True)
                if LDW_SKIP and c > 0:
                    inst.ins.ldweights = False
                nc.scalar.activation(tg[:, sl], p2[:], Tanh)
            k = 0
            for i in range(3):
                for c, sl in chunks:
                    p3 = pstile(f"acc{2 + k % 2}{(k // 2) % 2}")
                    inst = nc.tensor.matmul(p3, gwv, tp_vv[:, i, sl],
                                            start=True, stop=True)
                    if LDW_SKIP and k > 0:
                        inst.ins.ldweights = False
                    nc.scalar.copy(gvv[:, i, sl], p3[:])
                    k += 1
            tgv = tg[:].rearrange("p (s f) -> p s f", s=1)
            nc.vector.tensor_mul(gatedv, gvv,
                                 tgv.to_broadcast((128, 3, BC)))
            return tanh_s, gated, gatedv

        def factors(prefix, s_src, v_slices, Min, U, blk):
            """fused l1/l2 linears -> f1_all/f2_all/f2x + stage f1 to DRAM."""
            f1_all = acts.tile([128, 4 * BC], f16, tag="f1_all", name="f1_all")
            f2_all = acts.tile([128, 4 * BC], f16, tag="f2_all", name="f2_all")
            f1v = f1_all[:].rearrange("p (s f) -> p s f", s=4)
            f2v = f2_all[:].rearrange("p (s f) -> p s f", s=4)
            w1s, w1v, w2s, w2v = prefix
            dup = U == 64
            srcs = [s_src] + list(v_slices)
            chunks = [(c, slice(c * CH, (c + 1) * CH)) for c in range(NCH)]
            k = 0
            # f1 quantities first so DRAM staging can start early
            for j in range(4):
                w = w1s if j == 0 else w1v
                x = srcs[j]
                for ci, (c, sl) in enumerate(chunks):
                    p = pstile(f"acc{k % 4}{(k // 4) % 2}")
                    inst = nc.tensor.matmul(p[:U], w[:Min, :U], x[:, sl],
                                            start=True, stop=True)
                    if LDW_SKIP and ci > 0:
                        inst.ins.ldweights = False
                    if k % 2 == 0:
                        nc.scalar.copy(f1v[:U, j, sl], p[:U])
                    else:
                        nc.vector.tensor_copy(f1v[:U, j, sl], p[:U])
                    k += 1
            nc.sync.dma_start(f1d[blk][:U, :], f1_all[:U, :])
            # f2 quantities
            for j in range(4):
                w = w2s if j == 0 else w2v
                x = srcs[j]
                for ci, (c, sl) in enumerate(chunks):
                    p = pstile(f"acc{k % 4}{(k // 4) % 2}")
                    inst = nc.tensor.matmul(p[:U], w[:Min, :U], x[:, sl],
                                            start=True, stop=True)
                    if LDW_SKIP and ci > 0:
                        inst.ins.ldweights = False
                    eng = nc.scalar.copy if k % 2 == 0 else nc.vector.tensor_copy
                    eng(f2v[:U, j, sl], p[:U])
                    if dup:
                        eng2 = nc.vector.tensor_copy if k % 2 == 0 \
                            else nc.scalar.copy
                        eng2(f2v[64:128, j, sl], p[:64])
                    k += 1
            return f1_all, f2_all

        def _network():
            # ---- block 1 factors (plain l1/l2 on input) ----
            f1a, f2a = factors(
                (W["l1_s"], W["l1_v"], W["l2_s"], W["l2_v"]),
                sT, vT, 64, 64, "b1")

            def w1tile(kt):
                wv = wb1[:].rearrange("p (k t w) -> p k t w", k=32, t=4)
                return tuple(wv[:, kt, t, :] for t in range(4))

            tp_s, tp_v, tp_vv = tp_block("b1", 64, 32, f1a, f2a, w1tile)
            tanh_s, gated, gatedv = gate(W["g1_ws"], W["g1_wg"], W["g1_wv"],
                                         tp_s, tp_vv)

            # ---- block 2 factors (fused b1-out + b2-l1/l2) ----
            gsl = [gated[:].rearrange("p (s f) -> p s f", s=3)[:, i, :]
                   for i in range(3)]
            f1b, f2b = factors(
                (W["f2_l1_s"], W["f2_l1_v"], W["f2_l2_s"], W["f2_l2_v"]),
                tanh_s, gsl, 128, 128, "b2")

            wgrp = {}

            def w2tile(kt):
                g = kt // G_W
                if g not in wgrp:
                    wt = wstream.tile([128, G_W * 4 * 128], f16, tag="w2g",
                                      name="w2g")
                    nc.scalar.dma_start(
                        wt[:], wtp2_d[:, g * G_W * 512:(g + 1) * G_W * 512])
                    wgrp[g] = wt
                wv = wgrp[g][:].rearrange("p (k t w) -> p k t w", k=G_W, t=4)
                return tuple(wv[:, kt % G_W, t, :] for t in range(4))

            tp_s2, tp_v2, tp_vv2 = tp_block("b2", 128, 128, f1b, f2b, w2tile)
            tanh_s2, gated2, gatedv2 = gate(W["g2_ws"], W["g2_wg"], W["g2_wv"],
                                            tp_s2, tp_vv2)

            # ---- final fused linears -> out ----
            fo_a = acts.tile([128, BC], f32, tag="fo_a", name="fo_a")
            fo_b = acts.tile([128, BC], f32, tag="fo_b", name="fo_b")
            g2 = gated2[:].rearrange("p (s f) -> p s f", s=3)
            outs = [(W["ff_s"], 0, fo_a, 0), (W["ff_v"], 1, fo_a, 64),
                    (W["ff_v"], 2, fo_b, 0), (W["ff_v"], 3, fo_b, 64)]
            k = 0
            for idx, (w, _, dst, r0) in enumerate(outs):
                for c in range(NCH):
                    sl = slice(c * CH, (c + 1) * CH)
                    x = tanh_s2[:, sl] if idx == 0 else g2[:, idx - 1, sl]
                    p = pstile(f"acc{k % 4}{(k // 4) % 2}")
                    inst = nc.tensor.matmul(p[:64], w[:, :64], x,
                                            start=True, stop=True)
                    if LDW_SKIP and (idx, c) not in ((0, 0), (1, 0)):
                        inst.ins.ldweights = False
                    k += 1
                    nc.scalar.copy(dst[r0:r0 + 64, sl], p[:64])
            nc.sync.dma_start(out_d[0:128, :], fo_a[:])
            nc.sync.dma_start(out_d[128:256, :], fo_b[:])

        if repeat > 1:
            with tc.For_i(0, repeat, 1):
                _network()
        else:
            _network()

    nc.finalize()
    return nc


def _host_prep(inputs):
    """Fold norm constants, fuse out-linears, reorder/cast TP weights."""
    hf = np.float16
    d = {}
    c64 = np.float32(1.0 / np.sqrt(64.0))
    c128 = np.float32(1.0 / np.sqrt(128.0))
    c_og = np.float32(TANH_GAIN / np.sqrt(128.0))

    def tp_cat(blk, M):
        c_tp = 1.0 / (M * np.sqrt(2.0))
        mats = []
        for nm, c in (("ss", c_tp), ("vv", c_tp / np.sqrt(3.0)),
                      ("sv", c_tp), ("vs", c_tp)):
            mats.append((inputs[f"{blk}_tp_{nm}"] * np.float32(c)))
        arr = np.stack(mats, axis=0)          # [path, u, v, w]
        arr = arr.transpose(2, 1, 0, 3)       # [v, u, path, w]
        if M == 128:
            return np.ascontiguousarray(arr.reshape(128, -1)).astype(hf)
        # b1: partition p = r*64+v handles u=2kt+r
        a = arr.reshape(64, 32, 2, 4, 128)     # [v, kt, r, path, w]
        a = a.transpose(2, 0, 1, 3, 4)         # [r, v, kt, path, w]
        return np.ascontiguousarray(a.reshape(128, -1)).astype(hf)

    d["wtp1"] = tp_cat("b1", 64)
    d["wtp2"] = tp_cat("b2", 128)

    d["l1_s"] = (inputs["b1_l1_w0"] * c64).astype(hf)
    d["l1_v"] = (inputs["b1_l1_w1"] * c64).astype(hf)
    d["l2_s"] = (inputs["b1_l2_w0"] * c64).astype(hf)
    d["l2_v"] = (inputs["b1_l2_w1"] * c64).astype(hf)

    for b in ("1", "2"):
        for nm in ("ws", "wg", "wv"):
            d[f"g{b}_{nm}"] = (inputs[f"b{b}_g_{nm}"] * c128).astype(hf)

    # fused block1-out @ block2-l1/l2 (both with their norm constants)
    o0 = inputs["b1_o_w0"] * c_og
    o1 = inputs["b1_o_w1"] * c_og
    d["f2_l1_s"] = (o0 @ (inputs["b2_l1_w0"] * c128)).astype(hf)
    d["f2_l1_v"] = (o1 @ (inputs["b2_l1_w1"] * c128)).astype(hf)
    d["f2_l2_s"] = (o0 @ (inputs["b2_l2_w0"] * c128)).astype(hf)
    d["f2_l2_v"] = (o1 @ (inputs["b2_l2_w1"] * c128)).astype(hf)
    # fused block2-out @ final
    o0 = inputs["b2_o_w0"] * c_og
    o1 = inputs["b2_o_w1"] * c_og
    d["ff_s"] = (o0 @ (inputs["f_w0"] * c128)).astype(hf)
    d["ff_v"] = (o1 @ (inputs["f_w1"] * c128)).astype(hf)
    return d


def _shard_inputs(x, w):
    sT_full = np.ascontiguousarray(x[:, :64].T).astype(np.float16)
    v_full = x[:, 64:].reshape(B, 64, 3)
    vT_full = np.ascontiguousarray(
        v_full.transpose(2, 1, 0)).astype(np.float16)            # [3, 64, B]
    in_maps = []
    for c in range(N_CORES):
        bs = slice(c * BC, (c + 1) * BC)
        m = dict(w)
        m["s0"] = np.ascontiguousarray(sT_full[:, bs])
        m["v0"] = np.ascontiguousarray(vT_full[:, :, bs]).reshape(192, BC)
        in_maps.append(m)
    return in_maps


def kernel(**inputs):
    from concourse.bass_utils import run_bass_kernel_spmd

    x = np.asarray(inputs["x"], dtype=np.float32)
    w = _host_prep({k: np.asarray(v, dtype=np.float32)
                    for k, v in inputs.items() if k != "x"})

    if "nc" not in _CACHE:
        _CACHE["nc"] = _build_program()
    nc = _CACHE["nc"]

    in_maps = _shard_inputs(x, w)
    res = run_bass_kernel_spmd(nc, in_maps, core_ids=list(range(N_CORES)))

    out = np.empty((B, 256), dtype=np.float32)
    for c in range(N_CORES):
        o = res.results[c]["out"]                                # [256, BC]
        bs = slice(c * BC, (c + 1) * BC)
        out[bs, :64] = o[:64].T
        v = o[64:].reshape(3, 64, BC)
        out[bs, 64:] = v.transpose(2, 1, 0).reshape(BC, 192)
    return out
